# revision 1
# baseline (speedup 1.0000x reference)
"""Trainium2 Bass kernel for CustomRBF forward:

    out[i] = w * exp(-gamma * ||X[i] - centroid||^2) + b

Hybrid dual-path design (per core, data-parallel over 8 cores):
  - DMA X in natural layout [128 samples (partitions), 128 feats (free)],
    16 sample-tiles (1 MB) per dma_start.
  - Half-groups of 8 tiles alternate between two compute paths so TensorE
    and VectorE each carry ~half the per-sample reduction load in parallel:
    * PE path: TensorE transpose each tile to PSUM [feat, sample]; ScalarE
      fused subtract+square (activation Square, bias=-c, per-partition =
      per-feature); TensorE fp32r matmul (squared tile stationary, [1,0]
      moving) reduces over partitions -> 2 PSUM columns per tile ([sum, 0])
      in a [128, 512] accumulator.
    * DVE path (natural layout): VectorE tensor_sub against a replicated
      centroid row, ScalarE Square, VectorE segmented tensor_reduce over
      the feature axis -> [128, 8] columns in an SBUF [128, 256] accumulator.
  - Finalize per accumulator: ScalarE Exp (scale=-gamma), VectorE
    tensor_scalar (*w + b), TensorE transpose-back in 128-col chunks,
    VectorE PSUM->SBUF copy, then one output DMA per run of consecutive
    tiles (each path owns interleaved 8-tile blocks).

Sharding: cores 0-6 take contiguous 125056-sample slices; core 7 takes the
last 125056 samples (overlapping core 6 by 448 samples so every core gets
exactly 977 full 128-sample tiles). The overlap is recomputed identically
and overwritten at gather time.

`repeats` re-emits the whole pipeline R times in one NEFF (same data, same
output) — used only for differential wall-clock timing of the steady state.
"""

import sys

sys.path.insert(0, "/opt/trn_rl_repo")

import numpy as np

D = 128          # feature dim
P = 128          # SBUF partitions
GAMMA = 1.0 / D
N_CORES = 8
TILES = 977      # 128-sample tiles per core
SHARD = TILES * P           # 125056
N_TOTAL = 1000000
GROUP = 16       # tiles per DMA (HW-verified; 32 also works in sim)
HGROUP = 8       # tiles per half-group
PO_TILES = 256   # tiles per accumulator

_NC_CACHE = {}


def _build(tiles=TILES, po_tiles=PO_TILES, repeats=1, group=GROUP,
           xin_bufs=4, y_bufs=3, tr_bufs=2, stage="full", pe_num=1, den=2,
           depth=1, vsq=False):
    from contextlib import ExitStack

    import concourse.tile as tile
    from concourse import bacc, mybir

    f32 = mybir.dt.float32
    f32r = mybir.dt.float32r
    Act = mybir.ActivationFunctionType
    Alu = mybir.AluOpType

    n = tiles * P
    nc = bacc.Bacc("TRN2", target_bir_lowering=False, debug=False,
                   num_devices=N_CORES)
    xh = nc.declare_dram_parameter("x", [n, D], f32, isOutput=False)
    negch = nc.declare_dram_parameter("negc", [P, 1], f32, isOutput=False)
    identh = nc.declare_dram_parameter("ident", [P, D], f32, isOutput=False)
    onesh = nc.declare_dram_parameter("ones", [P, 2], f32, isOutput=False)
    creph = nc.declare_dram_parameter("crep", [P, HGROUP * D], f32,
                                      isOutput=False)
    wh = nc.declare_dram_parameter("wvec", [P, 1], f32, isOutput=False)
    bh = nc.declare_dram_parameter("bvec", [P, 1], f32, isOutput=False)
    outh = nc.declare_dram_parameter("out", [n], f32, isOutput=True)

    x_v = xh[:, :].rearrange("(t p) k -> p t k", p=P)  # [128, tiles, 128]

    with ExitStack() as ctx:
        tc = ctx.enter_context(tile.TileContext(nc))
        singles = ctx.enter_context(tc.tile_pool(name="singles", bufs=1))
        xin = ctx.enter_context(tc.tile_pool(name="xin", bufs=xin_bufs))
        yp = ctx.enter_context(tc.tile_pool(name="y", bufs=y_bufs))
        dfp = ctx.enter_context(tc.tile_pool(name="df", bufs=y_bufs))
        vsp = ctx.enter_context(tc.tile_pool(name="vs", bufs=2))
        resp = ctx.enter_context(tc.tile_pool(name="res", bufs=2))
        rtp = ctx.enter_context(tc.tile_pool(name="rt", bufs=3))
        trp = ctx.enter_context(tc.tile_pool(name="tr", bufs=tr_bufs,
                                             space="PSUM"))
        pop = ctx.enter_context(tc.tile_pool(name="po", bufs=2, space="PSUM"))
        ttp = ctx.enter_context(tc.tile_pool(name="tt", bufs=2, space="PSUM"))

        negc_s = singles.tile([P, 1], f32)
        nc.sync.dma_start(out=negc_s, in_=negch[:, :])
        ident_s = singles.tile([P, D], f32)
        nc.sync.dma_start(out=ident_s, in_=identh[:, :])
        ones_s = singles.tile([P, 2], f32)
        nc.sync.dma_start(out=ones_s, in_=onesh[:, :])
        ones_r = singles.tile([P, 2], f32r)
        nc.vector.tensor_copy(out=ones_r, in_=ones_s)
        crep_s = singles.tile([P, HGROUP * D], f32)
        nc.sync.dma_start(out=crep_s, in_=creph[:, :])
        crep3 = crep_s.rearrange("p (t k) -> p t k", k=D)
        wv_s = singles.tile([P, 1], f32)
        nc.sync.dma_start(out=wv_s, in_=wh[:, :])
        bv_s = singles.tile([P, 1], f32)
        nc.sync.dma_start(out=bv_s, in_=bh[:, :])

        pe_acc = {"buf": None, "tiles": []}
        v_acc = {"buf": None, "tiles": []}

        def finalize(acc, kind):
            buf, tlist = acc["buf"], acc["tiles"]
            T = len(tlist)
            stride = 2 if kind == "pe" else 1
            C = stride * T
            res = resp.tile([P, 2 * po_tiles], f32, name="res", tag="res")
            nc.scalar.activation(out=res[:, :C], in_=buf[:, :C],
                                 func=Act.Exp, scale=-GAMMA, bias=0.0)
            nc.vector.tensor_scalar(out=res[:, :C], in0=res[:, :C],
                                    scalar1=wv_s[:, :], scalar2=bv_s[:, :],
                                    op0=Alu.mult, op1=Alu.add)
            c0 = 0
            while c0 < C:
                ncol = min(P, C - c0)
                nt = ncol // stride
                t0 = c0 // stride
                tt = ttp.tile([P, D], f32, name="tt", tag="tt")
                nc.tensor.transpose(out=tt[:ncol, :],
                                    in_=res[:, c0:c0 + ncol],
                                    identity=ident_s[:, :])
                rt = rtp.tile([P, D], f32, name="rt", tag="rt")
                nc.vector.tensor_copy(out=rt[:ncol, :], in_=tt[:ncol, :])
                if kind == "pe":
                    rtv = rt.rearrange("(t two) f -> t two f", two=2)
                else:
                    rtv = None
                # one DMA per run of consecutive global tiles
                li = 0
                while li < nt:
                    lj = li + 1
                    while (lj < nt
                           and tlist[t0 + lj] == tlist[t0 + lj - 1] + 1):
                        lj += 1
                    L = lj - li
                    tg = tlist[t0 + li]
                    src = (rtv[li:lj, 0, :] if kind == "pe"
                           else rt[li:lj, :])
                    dest = outh[tg * P:(tg + L) * P].rearrange(
                        "(t p) -> t p", p=P)
                    nc.sync.dma_start(out=dest, in_=src)
                    li = lj
                c0 += ncol
            acc["buf"] = None
            acc["tiles"] = []

        # deferral per path: each halfgroup's reduce stage is emitted
        # `depth` halfgroups later, so neither engine stream stalls on the
        # cross-engine square in between.
        pending = {"pe": [], "v": []}

        def flush(path, all_=False):
            q = pending[path]
            while q and (all_ or len(q) >= depth):
                q.pop(0)()

        def pe_half(xt, hg, ht, t_base):
            tr = trp.tile([P, HGROUP * D], f32, name="tr", tag="tr")
            for j in range(ht):
                nc.tensor.transpose(out=tr[:, j * D:(j + 1) * D],
                                    in_=xt[:, hg + j, :],
                                    identity=ident_s[:, :])
            y = yp.tile([P, HGROUP * D], f32r, name="y", tag="y")
            nc.scalar.activation(out=y[:, :ht * D], in_=tr[:, :ht * D],
                                 func=Act.Square, bias=negc_s[:, :],
                                 scale=1.0)
            if stage == "sq":
                return

            def back():
                for j in range(ht):
                    if pe_acc["buf"] is None:
                        pe_acc["buf"] = pop.tile([P, 2 * po_tiles], f32,
                                                 name="po", tag="po")
                        pe_acc["tiles"] = []
                    col = 2 * len(pe_acc["tiles"])
                    nc.tensor.matmul(out=pe_acc["buf"][:, col:col + 2],
                                     lhsT=y[:, j * D:(j + 1) * D],
                                     rhs=ones_r[:, :], start=True, stop=True)
                    pe_acc["tiles"].append(t_base + j)
                    if len(pe_acc["tiles"]) == po_tiles:
                        finalize(pe_acc, "pe")

            pending["pe"].append(back)

        def v_half(xt, hg, ht, t_base):
            df = dfp.tile([P, HGROUP, D], f32, name="df", tag="df")
            nc.vector.tensor_sub(out=df[:, :ht, :], in0=xt[:, hg:hg + ht, :],
                                 in1=crep3[:, :ht, :])
            if vsq:
                # keep the whole V chain on VectorE (no ACT round-trip)
                nc.vector.tensor_mul(out=df[:, :ht, :], in0=df[:, :ht, :],
                                     in1=df[:, :ht, :])
            else:
                nc.scalar.activation(out=df[:, :ht, :], in_=df[:, :ht, :],
                                     func=Act.Square, bias=0.0, scale=1.0)
            if stage == "sq":
                return

            def back():
                if v_acc["buf"] is None:
                    v_acc["buf"] = vsp.tile([P, po_tiles], f32, name="vs",
                                            tag="vs")
                    v_acc["tiles"] = []
                c0 = len(v_acc["tiles"])
                nc.vector.tensor_reduce(out=v_acc["buf"][:, c0:c0 + ht],
                                        in_=df[:, :ht, :],
                                        axis=mybir.AxisListType.X,
                                        op=Alu.add)
                v_acc["tiles"].extend(t_base + j for j in range(ht))
                if len(v_acc["tiles"]) + HGROUP > po_tiles:
                    finalize(v_acc, "v")

            pending["v"].append(back)

        for _rep in range(repeats):
            hg_idx = 0
            t_done = 0
            while t_done < tiles:
                gt = min(group, tiles - t_done)
                xt = xin.tile([P, group, D], f32, name="xt", tag="xt")
                nc.sync.dma_start(out=xt[:, :gt, :],
                                  in_=x_v[:, t_done:t_done + gt, :])
                hg = 0
                while hg < gt and stage != "dma":
                    ht = min(HGROUP, gt - hg)
                    is_pe = (((hg_idx + 1) * pe_num) // den
                             > (hg_idx * pe_num) // den)
                    if is_pe:
                        flush("pe")
                        pe_half(xt, hg, ht, t_done + hg)
                    else:
                        flush("v")
                        v_half(xt, hg, ht, t_done + hg)
                    hg_idx += 1
                    hg += ht
                t_done += gt
            flush("pe", all_=True)
            flush("v", all_=True)
            if pe_acc["buf"] is not None:
                finalize(pe_acc, "pe")
            if v_acc["buf"] is not None:
                finalize(v_acc, "v")
        if stage != "full":
            # keep the output tensor written so the NEFF has a producer
            dest = outh[0:P].rearrange("(t p) -> t p", p=P)
            nc.sync.dma_start(out=dest, in_=ident_s[0:1, :])

    nc.finalize()
    return nc


def _get_nc(tiles=TILES):
    if tiles not in _NC_CACHE:
        _NC_CACHE[tiles] = _build(tiles)
    return _NC_CACHE[tiles]


def _make_const_inputs(centroid, w, b):
    centroid = np.asarray(centroid, dtype=np.float32).reshape(D)
    w = np.asarray(w, dtype=np.float32).reshape(-1)[0]
    b = np.asarray(b, dtype=np.float32).reshape(-1)[0]
    return {
        "negc": (-centroid).reshape(P, 1).copy(),
        "ident": np.eye(P, dtype=np.float32),
        "ones": np.tile(np.array([1.0, 0.0], dtype=np.float32), (P, 1)),
        "crep": np.tile(np.tile(centroid, HGROUP), (P, 1)),
        "wvec": np.full((P, 1), w, dtype=np.float32),
        "bvec": np.full((P, 1), b, dtype=np.float32),
    }


def kernel(X, centroid, w, b, _trace=False, _trace_kwargs=None):
    from concourse.bass_utils import run_bass_kernel_spmd

    X = np.asarray(X)
    assert X.shape == (N_TOTAL, D), X.shape
    if X.dtype != np.float32:
        X = X.astype(np.float32)

    consts = _make_const_inputs(centroid, w, b)
    starts = [i * SHARD for i in range(N_CORES - 1)] + [N_TOTAL - SHARD]
    in_maps = [dict(consts, x=X[s:s + SHARD]) for s in starts]

    nc = _get_nc()
    kw = {}
    if _trace:
        kw = dict(trace=True, **(_trace_kwargs or {}))
    res = run_bass_kernel_spmd(nc, in_maps, list(range(N_CORES)), **kw)

    out = np.empty(N_TOTAL, dtype=np.float32)
    for i, s in enumerate(starts):
        out[s:s + SHARD] = res.results[i]["out"]
    if _trace:
        return out, res
    return out



# revision 4
# speedup vs baseline: 1071.3247x; 1071.3247x over previous
"""Trainium2 Bass kernel for CustomRBF forward:

    out[i] = w * exp(-gamma * ||X[i] - centroid||^2) + b

Design (data-parallel over 8 cores, 125000 samples each — exact split):

  Host prep: X is transposed to [128 feats, N] and cast to bf16, so each
  core's shard is [128, 125000] with features on partitions and samples
  along the free axis (contiguous per partition -> line-rate DMA), padded
  with zeros to 245*512 columns.

  Per 512-sample chunk j (245 chunks):
    - square+subtract: ScalarE activation Square with per-partition bias
      -c (partition = feature), OR a DVE path (tensor_scalar add -c, then
      in-place tensor_mul) -> y_j = (x-c)^2 in bf16. DMA groups of 8
      chunks alternate between the two engines to balance load.
    - reduce over features (= partitions) on TensorE: one matmul per chunk
      with a sliding "ones in column r" stationary (lhsT = erow slice), so
      PSUM row r = j%128 accumulates chunk j's per-sample sums. 128 chunks
      accumulate into one [128, 512] PSUM bank (rows = chunks, cols =
      samples) whose flattened layout IS the output sample order.
  Two PSUM accumulation tiles cover 245 chunks; each is finalized with
  ScalarE Exp(scale=-gamma) PSUM->SBUF, VectorE w*K+b, and one contiguous
  output DMA.

Engine budget per core: DMA-in 31.4 MB bf16 ~ 88 us (the wall), ScalarE
~55 us, VectorE ~55 us, TensorE ~55-75 us.
"""

import sys

sys.path.insert(0, "/opt/trn_rl_repo")

import numpy as np

D = 128            # feature dim = SBUF partitions
P = 128
GAMMA = 1.0 / D
N_CORES = 8
N_TOTAL = 1000000
S = N_TOTAL // N_CORES      # 125000 samples per core
CHUNK = 512                 # samples per matmul chunk (one PSUM bank)
NCH = (S + CHUNK - 1) // CHUNK   # 245
S_PAD = NCH * CHUNK         # 125440
DMA_CH = 8                  # chunks per input DMA (8 KB/partition)
ACC_ROWS = P                # chunks per PSUM accumulation tile

_NC_CACHE = {}


def _build(repeats=1, stage="full", dve_num=16, dve_den=31, dma_ch=DMA_CH,
           xin_bufs=3, y_bufs=3):
    """dve_num/dve_den: fraction of DMA groups whose square runs on DVE."""
    from contextlib import ExitStack

    import concourse.tile as tile
    from concourse import bacc, mybir

    f32 = mybir.dt.float32
    bf16 = mybir.dt.bfloat16
    Act = mybir.ActivationFunctionType
    Alu = mybir.AluOpType

    nc = bacc.Bacc("TRN2", target_bir_lowering=False, debug=False,
                   num_devices=N_CORES)
    xh = nc.declare_dram_parameter("xt", [P, S_PAD], bf16, isOutput=False)
    negch = nc.declare_dram_parameter("negc", [P, 1], f32, isOutput=False)
    erowh = nc.declare_dram_parameter("erow", [P, 2 * P], bf16, isOutput=False)
    wh = nc.declare_dram_parameter("wvec", [P, 1], f32, isOutput=False)
    bh = nc.declare_dram_parameter("bvec", [P, 1], f32, isOutput=False)
    outh = nc.declare_dram_parameter("out", [S], f32, isOutput=True)

    n_groups = (NCH + dma_ch - 1) // dma_ch

    with ExitStack() as ctx:
        tc = ctx.enter_context(tile.TileContext(nc))
        singles = ctx.enter_context(tc.tile_pool(name="singles", bufs=1))
        xin = ctx.enter_context(tc.tile_pool(name="xin", bufs=xin_bufs))
        yp = ctx.enter_context(tc.tile_pool(name="y", bufs=y_bufs))
        finp = ctx.enter_context(tc.tile_pool(name="fin", bufs=2))
        accp = ctx.enter_context(tc.tile_pool(name="acc", bufs=2,
                                              space="PSUM"))

        negc_s = singles.tile([P, 1], f32)
        nc.sync.dma_start(out=negc_s, in_=negch[:, :])
        erow_s = singles.tile([P, 2 * P], bf16)
        nc.sync.dma_start(out=erow_s, in_=erowh[:, :])
        wv_s = singles.tile([P, 1], f32)
        nc.sync.dma_start(out=wv_s, in_=wh[:, :])
        bv_s = singles.tile([P, 1], f32)
        nc.sync.dma_start(out=bv_s, in_=bh[:, :])

        def finalize(acc, base_chunk, nch_in):
            """Exp + w*K+b + output DMA for one PSUM accumulation tile."""
            res = finp.tile([P, CHUNK], f32, name="res", tag="res")
            nc.scalar.activation(out=res[:nch_in, :], in_=acc[:nch_in, :],
                                 func=Act.Exp, scale=-GAMMA, bias=0.0)
            nc.vector.tensor_scalar(out=res[:nch_in, :], in0=res[:nch_in, :],
                                    scalar1=wv_s[:nch_in, :],
                                    scalar2=bv_s[:nch_in, :],
                                    op0=Alu.mult, op1=Alu.add)
            base = base_chunk * CHUNK
            nfull = min(nch_in, (S - base) // CHUNK)
            if nfull > 0:
                dest = outh[base:base + nfull * CHUNK].rearrange(
                    "(j n) -> j n", n=CHUNK)
                nc.sync.dma_start(out=dest, in_=res[:nfull, :])
            rem = min(S - base - nfull * CHUNK, CHUNK)
            if rem > 0 and nfull < nch_in:
                tb = base + nfull * CHUNK
                dest_t = outh[tb:tb + rem].rearrange("(j n) -> j n", n=rem)
                nc.sync.dma_start(out=dest_t,
                                  in_=res[nfull:nfull + 1, :rem])

        for _rep in range(repeats):
            acc = None
            acc_base = 0
            for g in range(n_groups):
                j0 = g * dma_ch
                gch = min(dma_ch, NCH - j0)
                fd = gch * CHUNK
                xt = xin.tile([P, dma_ch * CHUNK], bf16, name="xt", tag="xt")
                nc.sync.dma_start(out=xt[:, :fd],
                                  in_=xh[:, j0 * CHUNK:j0 * CHUNK + fd])
                if stage == "dma":
                    continue
                y = yp.tile([P, dma_ch * CHUNK], bf16, name="y", tag="y")
                is_dve = ((g + 1) * dve_num) // dve_den \
                    > (g * dve_num) // dve_den
                if is_dve:
                    nc.vector.tensor_scalar(out=y[:, :fd], in0=xt[:, :fd],
                                            scalar1=negc_s[:, :],
                                            scalar2=None, op0=Alu.add)
                    nc.vector.tensor_mul(out=y[:, :fd], in0=y[:, :fd],
                                         in1=y[:, :fd])
                else:
                    nc.scalar.activation(out=y[:, :fd], in_=xt[:, :fd],
                                         func=Act.Square, bias=negc_s[:, :],
                                         scale=1.0)
                if stage == "sq":
                    continue
                for lj in range(gch):
                    j = j0 + lj
                    r = j % ACC_ROWS
                    if r == 0:
                        if acc is not None:
                            finalize(acc, acc_base, ACC_ROWS)
                        acc = accp.tile([P, CHUNK], f32, name="acc",
                                        tag="acc")
                        acc_base = j
                    nc.tensor.matmul(out=acc[:, :],
                                     lhsT=erow_s[:, P - r:2 * P - r],
                                     rhs=y[:, lj * CHUNK:(lj + 1) * CHUNK],
                                     start=(r == 0),
                                     stop=(r == ACC_ROWS - 1 or j == NCH - 1))
            if stage == "full" and acc is not None:
                finalize(acc, acc_base, NCH - acc_base)
            if stage != "full":
                # keep the output tensor written so the NEFF has a producer
                dest = outh[0:P].rearrange("(j n) -> j n", n=P)
                nc.sync.dma_start(out=dest, in_=erow_s[0:1, 0:P])

    nc.finalize()
    return nc


def _get_nc():
    if "v2" not in _NC_CACHE:
        _NC_CACHE["v2"] = _build()
    return _NC_CACHE["v2"]


def _make_const_inputs(centroid, w, b):
    import ml_dtypes

    bf = ml_dtypes.bfloat16
    centroid = np.asarray(centroid, dtype=np.float32).reshape(D)
    w = np.asarray(w, dtype=np.float32).reshape(-1)[0]
    b = np.asarray(b, dtype=np.float32).reshape(-1)[0]
    erow = np.zeros((P, 2 * P), dtype=bf)
    erow[:, P] = 1.0
    return {
        "negc": (-centroid).reshape(P, 1).copy(),
        "erow": erow,
        "wvec": np.full((P, 1), w, dtype=np.float32),
        "bvec": np.full((P, 1), b, dtype=np.float32),
    }


def _make_x_shards(X):
    """[N, D] f32 -> per-core [P, S_PAD] bf16 (transposed, zero-padded)."""
    import ml_dtypes

    bf = ml_dtypes.bfloat16
    XT = np.ascontiguousarray(X.T).astype(bf)     # [128, N]
    shards = []
    for i in range(N_CORES):
        sh = np.zeros((P, S_PAD), dtype=bf)
        sh[:, :S] = XT[:, i * S:(i + 1) * S]
        shards.append(sh)
    return shards


def kernel(X, centroid, w, b, _trace=False, _trace_kwargs=None):
    from concourse.bass_utils import run_bass_kernel_spmd

    X = np.asarray(X)
    assert X.shape == (N_TOTAL, D), X.shape
    if X.dtype != np.float32:
        X = X.astype(np.float32)

    consts = _make_const_inputs(centroid, w, b)
    in_maps = [dict(consts, xt=sh) for sh in _make_x_shards(X)]

    nc = _get_nc()
    kw = {}
    if _trace:
        kw = dict(trace=True, **(_trace_kwargs or {}))
    res = run_bass_kernel_spmd(nc, in_maps, list(range(N_CORES)), **kw)

    out = np.empty(N_TOTAL, dtype=np.float32)
    for i in range(N_CORES):
        out[i * S:(i + 1) * S] = res.results[i]["out"]
    if _trace:
        return out, res
    return out


# revision 15
# speedup vs baseline: 1081.4053x; 1.0094x over previous
"""Trainium2 Bass kernel for CustomRBF forward:

    out[i] = w * exp(-gamma * ||X[i] - centroid||^2) + b

Design (data-parallel over 8 cores, 125000 samples each — exact split):

  Host prep: X is transposed to [128 feats, N] and cast to bf16, so each
  core's shard is [128, 125000] with features on partitions and samples
  along the free axis (contiguous per partition -> line-rate DMA), padded
  with zeros to 245*512 columns.

  Per 512-sample chunk j (245 chunks):
    - square+subtract: ScalarE activation Square with per-partition bias
      -c (partition = feature), OR a DVE path (tensor_scalar add -c, then
      in-place tensor_mul) -> y_j = (x-c)^2 in bf16. DMA groups of 8
      chunks alternate between the two engines to balance load.
    - reduce over features (= partitions) on TensorE: one matmul per chunk
      with a sliding "ones in column r" stationary (lhsT = erow slice), so
      PSUM row r = j%128 accumulates chunk j's per-sample sums. 128 chunks
      accumulate into one [128, 512] PSUM bank (rows = chunks, cols =
      samples) whose flattened layout IS the output sample order.
  Two PSUM accumulation tiles cover 245 chunks; each is finalized with
  ScalarE Exp(scale=-gamma) PSUM->SBUF, VectorE w*K+b, and one contiguous
  output DMA.

Engine budget per core: DMA-in 31.4 MB bf16 ~ 88 us (the wall), ScalarE
~55 us, VectorE ~55 us, TensorE ~55-75 us.
"""

import sys

sys.path.insert(0, "/opt/trn_rl_repo")

import numpy as np

D = 128            # feature dim = SBUF partitions
P = 128
GAMMA = 1.0 / D
N_CORES = 8
N_TOTAL = 1000000
S = N_TOTAL // N_CORES      # 125000 samples per core
CHUNK = 512                 # samples per matmul chunk (one PSUM bank)
NCH = (S + CHUNK - 1) // CHUNK   # 245
S_PAD = NCH * CHUNK         # 125440
DMA_CH = 8                  # chunks per input DMA (8 KB/partition)
ACC_ROWS = P                # chunks per PSUM accumulation tile

_NC_CACHE = {}


def _build(repeats=1, stage="full", sc_ch=4, dma_ch=DMA_CH,
           xin_bufs=8, y_bufs=8, d_bufs=4, out_gpsimd=True):
    """sc_ch: chunks per group squared on ScalarE (rest go to the DVE)."""
    from contextlib import ExitStack

    import concourse.tile as tile
    from concourse import bacc, mybir

    f32 = mybir.dt.float32
    bf16 = mybir.dt.bfloat16
    Act = mybir.ActivationFunctionType
    Alu = mybir.AluOpType

    nc = bacc.Bacc("TRN2", target_bir_lowering=False, debug=False,
                   num_devices=N_CORES)
    xh = nc.declare_dram_parameter("xt", [P, S_PAD], bf16, isOutput=False)
    negch = nc.declare_dram_parameter("negc", [P, 1], f32, isOutput=False)
    erowh = nc.declare_dram_parameter("erow", [P, 2 * P], bf16, isOutput=False)
    wh = nc.declare_dram_parameter("wvec", [P, 1], f32, isOutput=False)
    bh = nc.declare_dram_parameter("bvec", [P, 1], f32, isOutput=False)
    outh = nc.declare_dram_parameter("out", [S], f32, isOutput=True)

    group_sizes = [dma_ch] * ((NCH - 5) // dma_ch) + [4, 1]
    assert sum(group_sizes) == NCH, group_sizes

    with ExitStack() as ctx:
        tc = ctx.enter_context(tile.TileContext(nc))
        singles = ctx.enter_context(tc.tile_pool(name="singles", bufs=1))
        xin = ctx.enter_context(tc.tile_pool(name="xin", bufs=xin_bufs))
        yp = ctx.enter_context(tc.tile_pool(name="y", bufs=y_bufs))
        dp = ctx.enter_context(tc.tile_pool(name="d", bufs=d_bufs))
        finp = ctx.enter_context(tc.tile_pool(name="fin", bufs=2))
        accp = ctx.enter_context(tc.tile_pool(name="acc", bufs=2,
                                              space="PSUM"))

        negc_s = singles.tile([P, 1], f32)
        nc.sync.dma_start(out=negc_s, in_=negch[:, :])
        erow_s = singles.tile([P, 2 * P], bf16)
        nc.sync.dma_start(out=erow_s, in_=erowh[:, :])
        wv_s = singles.tile([P, 1], f32)
        nc.sync.dma_start(out=wv_s, in_=wh[:, :])
        bv_s = singles.tile([P, 1], f32)
        nc.sync.dma_start(out=bv_s, in_=bh[:, :])

        out_eng = nc.gpsimd if out_gpsimd else nc.sync

        def finalize(acc, base_chunk, nch_in):
            """Exp + w*K+b + output DMA for one PSUM accumulation tile."""
            res = finp.tile([P, CHUNK], f32, name="res", tag="res")
            nc.scalar.activation(out=res[:nch_in, :], in_=acc[:nch_in, :],
                                 func=Act.Exp, scale=-GAMMA, bias=0.0)
            nc.vector.tensor_scalar(out=res[:nch_in, :], in0=res[:nch_in, :],
                                    scalar1=wv_s[:nch_in, :],
                                    scalar2=bv_s[:nch_in, :],
                                    op0=Alu.mult, op1=Alu.add)
            base = base_chunk * CHUNK
            nfull = min(nch_in, (S - base) // CHUNK)
            if nfull > 0:
                dest = outh[base:base + nfull * CHUNK].rearrange(
                    "(j n) -> j n", n=CHUNK)
                out_eng.dma_start(out=dest, in_=res[:nfull, :])
            rem = min(S - base - nfull * CHUNK, CHUNK)
            if rem > 0 and nfull < nch_in:
                tb = base + nfull * CHUNK
                dest_t = outh[tb:tb + rem].rearrange("(j n) -> j n", n=rem)
                out_eng.dma_start(out=dest_t,
                                  in_=res[nfull:nfull + 1, :rem])

        for _rep in range(repeats):
            acc = None
            acc_base = 0
            jnext = 0
            for g, gch in enumerate(group_sizes):
                j0 = jnext
                jnext = j0 + gch
                sc = min(sc_ch, gch)          # chunks on the ScalarE path
                dv = gch - sc                 # chunks on the DVE path
                halves = []                   # (y_tile, n_chunks) in order
                xt_a = xin.tile([P, sc_ch * CHUNK], bf16, name="xa", tag="xa")
                nc.sync.dma_start(
                    out=xt_a[:, :sc * CHUNK],
                    in_=xh[:, j0 * CHUNK:(j0 + sc) * CHUNK])
                if dv > 0:
                    xt_b = xin.tile([P, (dma_ch - sc_ch) * CHUNK], bf16,
                                    name="xb", tag="xb")
                    nc.sync.dma_start(
                        out=xt_b[:, :dv * CHUNK],
                        in_=xh[:, (j0 + sc) * CHUNK:(j0 + gch) * CHUNK])
                if stage == "dma":
                    continue
                y_a = yp.tile([P, sc_ch * CHUNK], bf16, name="ya", tag="ya")
                nc.scalar.activation(out=y_a[:, :sc * CHUNK],
                                     in_=xt_a[:, :sc * CHUNK],
                                     func=Act.Square, bias=negc_s[:, :],
                                     scale=1.0)
                halves.append((y_a, sc))
                if dv > 0:
                    d = dp.tile([P, (dma_ch - sc_ch) * CHUNK], bf16,
                                name="d", tag="d")
                    nc.vector.tensor_scalar(out=d[:, :dv * CHUNK],
                                            in0=xt_b[:, :dv * CHUNK],
                                            scalar1=negc_s[:, :],
                                            scalar2=None, op0=Alu.add)
                    y_b = yp.tile([P, (dma_ch - sc_ch) * CHUNK], bf16,
                                  name="yb", tag="yb")
                    nc.vector.tensor_mul(out=y_b[:, :dv * CHUNK],
                                         in0=d[:, :dv * CHUNK],
                                         in1=d[:, :dv * CHUNK])
                    halves.append((y_b, dv))
                if stage == "sq":
                    continue
                lj = 0
                for y, hch in halves:
                    for hj in range(hch):
                        j = j0 + lj
                        r = j % ACC_ROWS
                        if r == 0:
                            if acc is not None:
                                finalize(acc, acc_base, ACC_ROWS)
                            acc = accp.tile([P, CHUNK], f32, name="acc",
                                            tag="acc")
                            acc_base = j
                        nc.tensor.matmul(out=acc[:, :],
                                         lhsT=erow_s[:, P - r:2 * P - r],
                                         rhs=y[:, hj * CHUNK:(hj + 1) * CHUNK],
                                         start=(r == 0),
                                         stop=(r == ACC_ROWS - 1
                                               or j == NCH - 1))
                        lj += 1
            if stage == "full" and acc is not None:
                finalize(acc, acc_base, NCH - acc_base)
            if stage != "full":
                # keep the output tensor written so the NEFF has a producer
                dest = outh[0:1].rearrange("(j n) -> j n", n=1)
                nc.sync.dma_start(out=dest, in_=wv_s[0:1, :])

    nc.finalize()
    return nc


def _get_nc():
    if "v2" not in _NC_CACHE:
        _NC_CACHE["v2"] = _build()
    return _NC_CACHE["v2"]


def _make_const_inputs(centroid, w, b):
    import ml_dtypes

    bf = ml_dtypes.bfloat16
    centroid = np.asarray(centroid, dtype=np.float32).reshape(D)
    w = np.asarray(w, dtype=np.float32).reshape(-1)[0]
    b = np.asarray(b, dtype=np.float32).reshape(-1)[0]
    erow = np.zeros((P, 2 * P), dtype=bf)
    erow[:, P] = 1.0
    return {
        "negc": (-centroid).reshape(P, 1).copy(),
        "erow": erow,
        "wvec": np.full((P, 1), w, dtype=np.float32),
        "bvec": np.full((P, 1), b, dtype=np.float32),
    }


def _make_x_shards(X):
    """[N, D] f32 -> per-core [P, S_PAD] bf16 (transposed, zero-padded)."""
    import ml_dtypes

    bf = ml_dtypes.bfloat16
    XT = np.ascontiguousarray(X.T).astype(bf)     # [128, N]
    shards = []
    for i in range(N_CORES):
        sh = np.zeros((P, S_PAD), dtype=bf)
        sh[:, :S] = XT[:, i * S:(i + 1) * S]
        shards.append(sh)
    return shards


def kernel(X, centroid, w, b, _trace=False, _trace_kwargs=None):
    from concourse.bass_utils import run_bass_kernel_spmd

    X = np.asarray(X)
    assert X.shape == (N_TOTAL, D), X.shape
    if X.dtype != np.float32:
        X = X.astype(np.float32)

    consts = _make_const_inputs(centroid, w, b)
    in_maps = [dict(consts, xt=sh) for sh in _make_x_shards(X)]

    nc = _get_nc()
    kw = {}
    if _trace:
        kw = dict(trace=True, **(_trace_kwargs or {}))
    res = run_bass_kernel_spmd(nc, in_maps, list(range(N_CORES)), **kw)

    out = np.empty(N_TOTAL, dtype=np.float32)
    for i in range(N_CORES):
        out[i * S:(i + 1) * S] = res.results[i]["out"]
    if _trace:
        return out, res
    return out


# revision 16
# speedup vs baseline: 1447.9873x; 1.3390x over previous
"""Trainium2 Bass kernel for CustomRBF forward:

    out[i] = w * exp(-gamma * ||X[i] - centroid||^2) + b

Design (data-parallel over 8 cores, 125000 samples each — exact split):

  Host prep: X is transposed to [128 feats, N] so each core's shard has
  features on partitions and samples along the free axis (contiguous per
  partition -> line-rate DMA), zero-padded to 245*512 columns. Chunks are
  packed into TWO arrays by compute path: ScalarE-path chunks in fp8-e4m3
  (ACT reads fp8 natively, computes in fp32) and DVE-path chunks in bf16
  (the DVE needs 16-bit for its 2x/4x perf modes). This cuts input DMA to
  ~23.6 MB/core.

  Per 512-sample chunk j (245 chunks), groups of 8 (4 fp8 + 4 bf16):
    - square+subtract: ScalarE activation Square with per-partition bias
      -c (partition = feature) on the fp8 half; tensor_scalar add -c then
      out-of-place tensor_mul on the bf16 half — both engines run every
      group in parallel -> y = (x-c)^2 in bf16.
    - reduce over features (= partitions) on TensorE: one matmul per chunk
      with a sliding "ones in column r" stationary (lhsT = erow slice), so
      PSUM row r = j%128 accumulates chunk j's per-sample sums. 128 chunks
      accumulate into one [128, 512] PSUM bank whose flattened layout IS
      the output sample order.
  Two PSUM accumulation tiles cover 245 chunks; each is finalized with
  ScalarE Exp(scale=-gamma) PSUM->SBUF, VectorE w*K+b, and a contiguous
  output DMA on the GpSimd (SWDGE) ring to keep the Sync ring free for
  input triggers.

Engine budget per core: ScalarE ~60 us, VectorE ~60 us, TensorE ~65 us,
DMA-in ~67 us; span ~= fill + compute + tail ~ 90 us.
"""

import sys

sys.path.insert(0, "/opt/trn_rl_repo")

import numpy as np

D = 128            # feature dim = SBUF partitions
P = 128
GAMMA = 1.0 / D
N_CORES = 8
N_TOTAL = 1000000
S = N_TOTAL // N_CORES      # 125000 samples per core
CHUNK = 512                 # samples per matmul chunk (one PSUM bank)
NCH = (S + CHUNK - 1) // CHUNK   # 245
S_PAD = NCH * CHUNK         # 125440
DMA_CH = 8                  # chunks per group
SC_CH = 4                   # chunks per group on the ScalarE (fp8) path
ACC_ROWS = P                # chunks per PSUM accumulation tile

GROUP_SIZES = [DMA_CH] * ((NCH - 5) // DMA_CH) + [4, 1]
assert sum(GROUP_SIZES) == NCH


def _path_layout(sc_ch=SC_CH):
    """Per group: (gch, sc, dv, sc_chunk_offset, dv_chunk_offset)."""
    lay = []
    o_sc = o_dv = 0
    for gch in GROUP_SIZES:
        sc = min(sc_ch, gch)
        dv = gch - sc
        lay.append((gch, sc, dv, o_sc, o_dv))
        o_sc += sc
        o_dv += dv
    return lay, o_sc, o_dv


_LAYOUT, N_SC, N_DV = _path_layout()

_NC_CACHE = {}


def _build(repeats=1, stage="full", xin_bufs=8, y_bufs=8, d_bufs=4,
           out_gpsimd=True, sc_f8=True):
    from contextlib import ExitStack

    import concourse.tile as tile
    from concourse import bacc, mybir

    f32 = mybir.dt.float32
    bf16 = mybir.dt.bfloat16
    f8 = mybir.dt.float8e4 if sc_f8 else mybir.dt.bfloat16
    Act = mybir.ActivationFunctionType
    Alu = mybir.AluOpType

    nc = bacc.Bacc("TRN2", target_bir_lowering=False, debug=False,
                   num_devices=N_CORES)
    x8h = nc.declare_dram_parameter("x8", [P, N_SC * CHUNK], f8,
                                    isOutput=False)
    xbh = nc.declare_dram_parameter("xb", [P, max(N_DV, 1) * CHUNK], bf16,
                                    isOutput=False)
    negch = nc.declare_dram_parameter("negc", [P, 1], f32, isOutput=False)
    erowh = nc.declare_dram_parameter("erow", [P, 2 * P], bf16, isOutput=False)
    wh = nc.declare_dram_parameter("wvec", [P, 1], f32, isOutput=False)
    bh = nc.declare_dram_parameter("bvec", [P, 1], f32, isOutput=False)
    outh = nc.declare_dram_parameter("out", [S], f32, isOutput=True)

    with ExitStack() as ctx:
        tc = ctx.enter_context(tile.TileContext(nc))
        singles = ctx.enter_context(tc.tile_pool(name="singles", bufs=1))
        xin = ctx.enter_context(tc.tile_pool(name="xin", bufs=xin_bufs))
        yp = ctx.enter_context(tc.tile_pool(name="y", bufs=y_bufs))
        dp = ctx.enter_context(tc.tile_pool(name="d", bufs=d_bufs))
        finp = ctx.enter_context(tc.tile_pool(name="fin", bufs=2))
        accp = ctx.enter_context(tc.tile_pool(name="acc", bufs=2,
                                              space="PSUM"))

        negc_s = singles.tile([P, 1], f32)
        nc.sync.dma_start(out=negc_s, in_=negch[:, :])
        erow_s = singles.tile([P, 2 * P], bf16)
        nc.sync.dma_start(out=erow_s, in_=erowh[:, :])
        wv_s = singles.tile([P, 1], f32)
        nc.sync.dma_start(out=wv_s, in_=wh[:, :])
        bv_s = singles.tile([P, 1], f32)
        nc.sync.dma_start(out=bv_s, in_=bh[:, :])

        out_eng = nc.gpsimd if out_gpsimd else nc.sync

        def finalize(acc, base_chunk, nch_in):
            """Exp + w*K+b + output DMA for one PSUM accumulation tile."""
            res = finp.tile([P, CHUNK], f32, name="res", tag="res")
            nc.scalar.activation(out=res[:nch_in, :], in_=acc[:nch_in, :],
                                 func=Act.Exp, scale=-GAMMA, bias=0.0)
            nc.vector.tensor_scalar(out=res[:nch_in, :], in0=res[:nch_in, :],
                                    scalar1=wv_s[:nch_in, :],
                                    scalar2=bv_s[:nch_in, :],
                                    op0=Alu.mult, op1=Alu.add)
            base = base_chunk * CHUNK
            nfull = min(nch_in, (S - base) // CHUNK)
            if nfull > 0:
                dest = outh[base:base + nfull * CHUNK].rearrange(
                    "(j n) -> j n", n=CHUNK)
                out_eng.dma_start(out=dest, in_=res[:nfull, :])
            rem = min(S - base - nfull * CHUNK, CHUNK)
            if rem > 0 and nfull < nch_in:
                tb = base + nfull * CHUNK
                dest_t = outh[tb:tb + rem].rearrange("(j n) -> j n", n=rem)
                out_eng.dma_start(out=dest_t,
                                  in_=res[nfull:nfull + 1, :rem])

        for _rep in range(repeats):
            acc = None
            acc_base = 0
            j0 = 0
            for (gch, sc, dv, o_sc, o_dv) in _LAYOUT:
                halves = []                   # (y_tile, n_chunks) in order
                xt_a = xin.tile([P, SC_CH * CHUNK], f8, name="xa", tag="xa")
                nc.sync.dma_start(
                    out=xt_a[:, :sc * CHUNK],
                    in_=x8h[:, o_sc * CHUNK:(o_sc + sc) * CHUNK])
                if dv > 0:
                    xt_b = xin.tile([P, (DMA_CH - SC_CH) * CHUNK], bf16,
                                    name="xb", tag="xb")
                    nc.sync.dma_start(
                        out=xt_b[:, :dv * CHUNK],
                        in_=xbh[:, o_dv * CHUNK:(o_dv + dv) * CHUNK])
                if stage == "dma":
                    j0 += gch
                    continue
                y_a = yp.tile([P, SC_CH * CHUNK], bf16, name="ya", tag="ya")
                nc.scalar.activation(out=y_a[:, :sc * CHUNK],
                                     in_=xt_a[:, :sc * CHUNK],
                                     func=Act.Square, bias=negc_s[:, :],
                                     scale=1.0)
                halves.append((y_a, sc))
                if dv > 0:
                    d = dp.tile([P, (DMA_CH - SC_CH) * CHUNK], bf16,
                                name="d", tag="d")
                    nc.vector.tensor_scalar(out=d[:, :dv * CHUNK],
                                            in0=xt_b[:, :dv * CHUNK],
                                            scalar1=negc_s[:, :],
                                            scalar2=None, op0=Alu.add)
                    y_b = yp.tile([P, (DMA_CH - SC_CH) * CHUNK], bf16,
                                  name="yb", tag="yb")
                    nc.vector.tensor_mul(out=y_b[:, :dv * CHUNK],
                                         in0=d[:, :dv * CHUNK],
                                         in1=d[:, :dv * CHUNK])
                    halves.append((y_b, dv))
                if stage == "sq":
                    j0 += gch
                    continue
                lj = 0
                for y, hch in halves:
                    for hj in range(hch):
                        j = j0 + lj
                        r = j % ACC_ROWS
                        if r == 0:
                            if acc is not None:
                                finalize(acc, acc_base, ACC_ROWS)
                            acc = accp.tile([P, CHUNK], f32, name="acc",
                                            tag="acc")
                            acc_base = j
                        nc.tensor.matmul(out=acc[:, :],
                                         lhsT=erow_s[:, P - r:2 * P - r],
                                         rhs=y[:, hj * CHUNK:(hj + 1) * CHUNK],
                                         start=(r == 0),
                                         stop=(r == ACC_ROWS - 1
                                               or j == NCH - 1))
                        lj += 1
                j0 += gch
            if stage == "full" and acc is not None:
                finalize(acc, acc_base, NCH - acc_base)
            if stage != "full":
                # keep the output tensor written so the NEFF has a producer
                dest = outh[0:1].rearrange("(j n) -> j n", n=1)
                nc.sync.dma_start(out=dest, in_=wv_s[0:1, :])

    nc.finalize()
    return nc


def _get_nc():
    if "v3" not in _NC_CACHE:
        _NC_CACHE["v3"] = _build()
    return _NC_CACHE["v3"]


def _make_const_inputs(centroid, w, b):
    import ml_dtypes

    bf = ml_dtypes.bfloat16
    centroid = np.asarray(centroid, dtype=np.float32).reshape(D)
    w = np.asarray(w, dtype=np.float32).reshape(-1)[0]
    b = np.asarray(b, dtype=np.float32).reshape(-1)[0]
    erow = np.zeros((P, 2 * P), dtype=bf)
    erow[:, P] = 1.0
    return {
        "negc": (-centroid).reshape(P, 1).copy(),
        "erow": erow,
        "wvec": np.full((P, 1), w, dtype=np.float32),
        "bvec": np.full((P, 1), b, dtype=np.float32),
    }


_SC_IDX = None
_DV_IDX = None


def _chunk_indices():
    global _SC_IDX, _DV_IDX
    if _SC_IDX is None:
        sc_idx, dv_idx = [], []
        j0 = 0
        for (gch, sc, dv, _, _) in _LAYOUT:
            sc_idx.extend(range(j0, j0 + sc))
            dv_idx.extend(range(j0 + sc, j0 + gch))
            j0 += gch
        _SC_IDX = np.array(sc_idx)
        _DV_IDX = np.array(dv_idx)
    return _SC_IDX, _DV_IDX


def _make_x_shards(X):
    """[N, D] f32 -> per-core {x8: [P, N_SC*CHUNK] f8, xb: bf16} arrays."""
    import ml_dtypes

    bf = ml_dtypes.bfloat16
    f8 = ml_dtypes.float8_e4m3
    sc_idx, dv_idx = _chunk_indices()
    XT = np.ascontiguousarray(X.T)                # [128, N] f32
    shards = []
    for i in range(N_CORES):
        sh = np.zeros((P, S_PAD), dtype=np.float32)
        sh[:, :S] = XT[:, i * S:(i + 1) * S]
        sh3 = sh.reshape(P, NCH, CHUNK)
        x8 = np.ascontiguousarray(
            sh3[:, sc_idx, :]).reshape(P, N_SC * CHUNK).astype(f8)
        xb = np.ascontiguousarray(
            sh3[:, dv_idx, :]).reshape(P, N_DV * CHUNK).astype(bf)
        shards.append({"x8": x8, "xb": xb})
    return shards


def kernel(X, centroid, w, b, _trace=False, _trace_kwargs=None):
    from concourse.bass_utils import run_bass_kernel_spmd

    X = np.asarray(X)
    assert X.shape == (N_TOTAL, D), X.shape
    if X.dtype != np.float32:
        X = X.astype(np.float32)

    consts = _make_const_inputs(centroid, w, b)
    in_maps = [dict(consts, **sh) for sh in _make_x_shards(X)]

    nc = _get_nc()
    kw = {}
    if _trace:
        kw = dict(trace=True, **(_trace_kwargs or {}))
    res = run_bass_kernel_spmd(nc, in_maps, list(range(N_CORES)), **kw)

    out = np.empty(N_TOTAL, dtype=np.float32)
    for i in range(N_CORES):
        out[i * S:(i + 1) * S] = res.results[i]["out"]
    if _trace:
        return out, res
    return out
